# revision 1
# baseline (speedup 1.0000x reference)
"""CityExpertMoE Trainium2 kernel.

Two-phase, 8-core design:
  Phase 1 (data-parallel over tokens): LayerNorm + router logits + top-2
  combine weights, computed in transposed [D, tok] layout.
  Host: gather tokens by expert id ("all-to-all dispatch").
  Phase 2 (expert-parallel): core e runs expert e's FFN
  (1024 -> 4096 GELU -> 1024, bf16 matmuls, fp32 accumulate) on its
  gathered tokens, scales rows by the combine weight.
  Host: scatter-add partial outputs + residual + b2 (top-2 weights sum to 1).
"""

import sys
import types

import numpy as np
import ml_dtypes

# If BASS_TRACE is set but the axon NTFF hook shim is absent, bass_utils
# would fail importing antenv.axon_hooks; register a no-op fallback.
try:
    import antenv.axon_hooks  # noqa: F401
except ImportError:
    _m = types.ModuleType("antenv.axon_hooks")
    _m._hook = None
    _m.set_axon_ntff_profile_hook = lambda h: setattr(_m, "_hook", h)
    _m.get_axon_ntff_profile_hook = lambda: _m._hook
    sys.modules["antenv.axon_hooks"] = _m
    try:
        import antenv
        antenv.axon_hooks = _m
    except ImportError:
        pass

import concourse.bass as bass
import concourse.mybir as mybir
import concourse.tile as tile
from concourse import bacc
from concourse import masks
from concourse.bass_utils import run_bass_kernel_spmd

F32 = mybir.dt.float32
BF16 = mybir.dt.bfloat16
AF = mybir.ActivationFunctionType
ALU = mybir.AluOpType

B, L, D, H, E, TOP_K = 4, 2048, 1024, 4096, 8, 2
T = B * L               # 8192 tokens total
N_CORES = 8
TC = T // N_CORES       # 1024 tokens per core in phase 1
KT = D // 128           # 8 k-tiles over D
HT = H // 128           # 32 k-tiles over H
LN_EPS = 1e-5
BLK = 512               # phase-2 token block

_cache: dict = {}
LAST_RESULTS: dict = {}


# ---------------------------------------------------------------- phase 1
def build_phase1(affine: bool):
    """LayerNorm + router top-2. x comes in twice (rows and transposed).

    Pass 1 (per 128-token tile): bn_stats fused mean/var, rsqrt via
    Sqrt+fast-reciprocal, one fused normalize op writing bf16 directly.
    Router: logits = r*(x @ gwg) - r*mu*colsum(gwg) + beta @ gate_w
    (exact), computed from the raw x^T upload; per-token terms applied in
    [token, E] layout where r and mu*r are per-partition scalars.
    Pass 2: batched top-2 renormalized weights over all tiles at once.
    """
    nc = bacc.Bacc("TRN2", target_bir_lowering=False, debug=False,
                   num_devices=N_CORES)
    xr_d = nc.dram_tensor("xr", [TC, D], F32, kind="ExternalInput").ap()
    xT_d = nc.dram_tensor("xT", [D, TC], F32, kind="ExternalInput").ap()
    gate_w = nc.dram_tensor("gate_w", [D, E], F32, kind="ExternalInput").ap()
    if affine:
        gr_d = nc.dram_tensor("gamma_r", [128, KT], F32, kind="ExternalInput").ap()
        br_d = nc.dram_tensor("beta_r", [128, KT], F32, kind="ExternalInput").ap()
        gb_d = nc.dram_tensor("gb", [128, D], F32, kind="ExternalInput").ap()
        bb_d = nc.dram_tensor("bb", [128, D], F32, kind="ExternalInput").ap()
    xn_o = nc.dram_tensor("xn", [TC, D], BF16, kind="ExternalOutput").ap()
    cw_o = nc.dram_tensor("cw", [TC, E], F32, kind="ExternalOutput").ap()

    NTT = TC // 128      # 128-token tiles
    NCH = TC // 512

    with tile.TileContext(nc) as tc:
        import contextlib
        with contextlib.ExitStack() as ctx:
            const = ctx.enter_context(tc.tile_pool(name="const", bufs=1))
            xin = ctx.enter_context(tc.tile_pool(name="xin", bufs=3))
            xnp = ctx.enter_context(tc.tile_pool(name="xnp", bufs=2))
            big = ctx.enter_context(tc.tile_pool(name="big", bufs=1))
            work = ctx.enter_context(tc.tile_pool(name="work", bufs=4))
            pers = ctx.enter_context(tc.tile_pool(name="pers", bufs=1))
            ps_r = ctx.enter_context(
                tc.tile_pool(name="ps_r", bufs=3, space="PSUM"))
            ps_l = ctx.enter_context(
                tc.tile_pool(name="ps_l", bufs=3, space="PSUM"))

            gw_sb = const.tile([128, KT, E], F32)
            nc.sync.dma_start(gw_sb[:], gate_w.rearrange("(k p) e -> p k e", p=128))
            epst = const.tile([128, 1], F32)
            nc.vector.memset(epst[:], LN_EPS)
            zerot = const.tile([128, 1], F32)
            nc.vector.memset(zerot[:], 0.0)
            ident8 = const.tile([8, 8], F32)
            masks.make_identity(nc, ident8[:])
            ones_col = const.tile([128, 1], F32)
            nc.vector.memset(ones_col[:], 1.0)
            ones_row = const.tile([1, 128], F32)
            nc.vector.memset(ones_row[:], 1.0)
            if affine:
                g_r = const.tile([128, KT], F32)
                nc.sync.dma_start(g_r[:], gr_d[:])
                b_r = const.tile([128, KT], F32)
                nc.sync.dma_start(b_r[:], br_d[:])
                gb = const.tile([128, D], F32)
                nc.sync.dma_start(gb[:], gb_d[:])
                bb = const.tile([128, D], F32)
                nc.sync.dma_start(bb[:], bb_d[:])
                gwg = const.tile([128, KT, E], F32)
                for k in range(KT):
                    nc.vector.tensor_scalar(gwg[:, k, :], gw_sb[:, k, :],
                                            g_r[:, k:k + 1], None, ALU.mult)
            else:
                gwg = gw_sb

            # B = colsum(gwg) as [128, 1, E]-broadcastable row; C0 likewise
            ps_b = ps_l.tile([1, E], F32, tag="lg", name="ps_b")
            for k in range(KT):
                nc.tensor.matmul(ps_b[:], ones_col[:], gwg[:, k, :],
                                 start=(k == 0), stop=(k == KT - 1))
            b_row = work.tile([1, E], F32, tag="b_row")
            nc.vector.tensor_copy(b_row[:], ps_b[:])
            ps_bb = ps_l.tile([128, E], F32, tag="lg", name="ps_bb")
            nc.tensor.matmul(ps_bb[:], ones_row[:], b_row[:],
                             start=True, stop=True)
            B_b = const.tile([128, E], F32)
            nc.vector.tensor_copy(B_b[:], ps_bb[:])
            if affine:
                ps_c = ps_l.tile([1, E], F32, tag="lg", name="ps_c")
                for k in range(KT):
                    bgw = work.tile([128, E], F32, tag="bgw")
                    nc.vector.tensor_scalar(bgw[:], gw_sb[:, k, :],
                                            b_r[:, k:k + 1], None, ALU.mult)
                    nc.tensor.matmul(ps_c[:], ones_col[:], bgw[:],
                                     start=(k == 0), stop=(k == KT - 1))
                c_row = work.tile([1, E], F32, tag="c_row")
                nc.vector.tensor_copy(c_row[:], ps_c[:])
                ps_cb = ps_l.tile([128, E], F32, tag="lg", name="ps_cb")
                nc.tensor.matmul(ps_cb[:], ones_row[:], c_row[:],
                                 start=True, stop=True)
                C0_b = const.tile([128, E], F32)
                nc.vector.tensor_copy(C0_b[:], ps_cb[:])

            # interleave rows (LN) and x^T token-chunks (router) so both
            # pipelines chase the single saturated DMA stream
            xT_sb = big.tile([128, KT, TC], F32)
            xT_r = xT_d.rearrange("(k p) t -> p k t", p=128)
            xr_tiles = [xin.tile([128, D], F32, tag="xr", name=f"xr_{t}",
                                 bufs=NTT) for t in range(NTT)]
            for half in range(2):
                for t in range(half * (NTT // 2), (half + 1) * (NTT // 2)):
                    nc.sync.dma_start(xr_tiles[t][:], xr_d[bass.ts(t, 128), :])
                csl = bass.ts(half, TC // 2)
                nc.sync.dma_start(xT_sb[:, :, csl], xT_r[:, :, csl])
            A_row = big.tile([8, TC], F32)
            for ch in range(NCH):
                ps = ps_r.tile([8, 512], F32, tag="A", name=f"A_{ch}")
                for k in range(KT):
                    nc.tensor.matmul(ps[:], gwg[:, k, :],
                                     xT_sb[:, k, bass.ts(ch, 512)],
                                     start=(k == 0), stop=(k == KT - 1))
                nc.vector.tensor_copy(A_row[:, bass.ts(ch, 512)], ps[:])

            # ---- pass 1: LN per tile ----
            xnb_all = big.tile([128, NTT, D], BF16)
            r_all = pers.tile([128, NTT], F32)
            mrn_all = pers.tile([128, NTT], F32)
            t1_all = pers.tile([128, NTT, E], F32)
            for t in range(NTT):
                tsl = bass.ts(t, 128)
                xr = xr_tiles[t]
                bst = work.tile([128, 2, 6], F32, tag="bst")
                for g in range(2):
                    nc.vector.bn_stats(bst[:, g, :], xr[:, bass.ts(g, 512)])
                mv = work.tile([128, 2], F32, tag="mv")
                nc.vector.bn_aggr(mv[:], bst[:])
                std = work.tile([128, 1], F32, tag="std")
                nc.scalar.activation(std[:], mv[:, 1:2], AF.Sqrt, bias=epst[:])
                scr = work.tile([128, 1], F32, tag="scr")
                nc.vector.reciprocal_approx_accurate(r_all[:, t:t + 1], std[:],
                                                     scr[:])
                nc.vector.tensor_scalar(mrn_all[:, t:t + 1], mv[:, 0:1],
                                        r_all[:, t:t + 1], -1.0,
                                        ALU.mult, ALU.mult)
                if affine:
                    xn = xnp.tile([128, D], F32, tag="xn")
                    nc.vector.tensor_scalar(xn[:], xr[:], mv[:, 0:1],
                                            r_all[:, t:t + 1],
                                            ALU.subtract, ALU.mult)
                    nc.vector.tensor_mul(xn[:], xn[:], gb[:])
                    nc.vector.tensor_add(xnb_all[:, t, :], xn[:], bb[:])
                else:
                    nc.scalar.activation(xnb_all[:, t, :], xr[:], AF.Identity,
                                         bias=mrn_all[:, t:t + 1],
                                         scale=r_all[:, t:t + 1])
                # per-tile slice of the router correction: t1 = r * A_t
                At_ps = ps_l.tile([128, E], F32, tag="lg", name=f"At_{t}")
                nc.tensor.transpose(At_ps[:], A_row[:, tsl], ident8[:])
                nc.vector.tensor_scalar(t1_all[:, t, :], At_ps[:],
                                        r_all[:, t:t + 1], None, ALU.mult)

            xn_r = xn_o.rearrange("(t p) d -> p t d", p=128)
            hN = NTT // 2
            nc.sync.dma_start(xn_r[:, 0:hN, :], xnb_all[:, 0:hN, :])
            nc.sync.dma_start(xn_r[:, hN:NTT, :], xnb_all[:, hN:NTT, :])

            # ---- pass 2: batched top-2 over [128, NTT, E] ----
            def bc_t(ap_2d):     # [128, NTT] -> [128, NTT, E] (0-step E)
                return ap_2d.to_broadcast((128, NTT, E))

            def bc_e(ap_2d):     # [128, E] -> [128, NTT, E] (0-step NTT)
                return ap_2d.rearrange("p (t e) -> p t e",
                                       t=1).to_broadcast((128, NTT, E))

            lg = pers.tile([128, NTT, E], F32)
            nc.vector.tensor_tensor(lg[:], bc_t(mrn_all[:]), bc_e(B_b[:]),
                                    ALU.mult)
            nc.vector.tensor_add(lg[:], lg[:], t1_all[:])
            if affine:
                nc.vector.tensor_tensor(lg[:], lg[:], bc_e(C0_b[:]), ALU.add)
            m1 = pers.tile([128, NTT], F32)
            nc.vector.reduce_max(m1[:], lg[:], axis=mybir.AxisListType.X)
            eq = pers.tile([128, NTT, E], F32)
            nc.vector.tensor_tensor(eq[:], lg[:], bc_t(m1[:]), ALU.is_equal)
            masked = pers.tile([128, NTT, E], F32)
            nc.vector.scalar_tensor_tensor(masked[:], eq[:], -1e30, lg[:],
                                           ALU.mult, ALU.add)
            m2 = pers.tile([128, NTT], F32)
            nc.vector.reduce_max(m2[:], masked[:], axis=mybir.AxisListType.X)
            ge = pers.tile([128, NTT, E], F32)
            nc.vector.tensor_tensor(ge[:], lg[:], bc_t(m2[:]), ALU.is_ge)
            sub = pers.tile([128, NTT, E], F32)
            nc.vector.tensor_tensor(sub[:], lg[:], bc_t(m1[:]), ALU.subtract)
            exps = pers.tile([128, NTT, E], F32)
            nc.scalar.activation(exps[:], sub[:], AF.Exp, bias=zerot[:])
            cw_raw = pers.tile([128, NTT, E], F32)
            nc.vector.tensor_mul(cw_raw[:], exps[:], ge[:])
            den = pers.tile([128, NTT], F32)
            nc.vector.reduce_sum(den[:], cw_raw[:], axis=mybir.AxisListType.X)
            rec = pers.tile([128, NTT], F32)
            scr2 = pers.tile([128, NTT], F32)
            nc.vector.reciprocal_approx_accurate(rec[:], den[:], scr2[:])
            cw_all = pers.tile([128, NTT, E], F32)
            nc.vector.tensor_tensor(cw_all[:], cw_raw[:], bc_t(rec[:]),
                                    ALU.mult)
            nc.sync.dma_start(cw_o.rearrange("(t p) e -> p t e", p=128),
                              cw_all[:])

    nc.compile()
    return nc


# ---------------------------------------------------------------- phase 2
def build_phase2(C: int):
    """Expert FFN on C gathered (padded) tokens; one expert per core.

    Blocks of 512 tokens; only the final block may be ragged (any size)."""
    blocks = [512] * (C // 512)
    r = C % 512
    if r:
        if r < 256 and blocks:
            blocks.pop()
            total = 512 + r
            first = ((total + 1) // 2 + 127) // 128 * 128
            blocks.extend([first, total - first])
        else:
            blocks.append(r)
    nc = bacc.Bacc("TRN2", target_bir_lowering=False, debug=False,
                   num_devices=N_CORES)
    xnT = nc.dram_tensor("xnT", [D, C], BF16, kind="ExternalInput").ap()
    w1 = nc.dram_tensor("w1", [D, H], BF16, kind="ExternalInput").ap()
    w2 = nc.dram_tensor("w2", [H, D], BF16, kind="ExternalInput").ap()
    b1r = nc.dram_tensor("b1r", [128, HT], F32, kind="ExternalInput").ap()
    CR = (C + 127) // 128
    cwr = nc.dram_tensor("cwr", [128, CR], F32, kind="ExternalInput").ap()
    y_o = nc.dram_tensor("y", [C, D], F32, kind="ExternalOutput").ap()

    with tile.TileContext(nc) as tc:
        import contextlib
        with contextlib.ExitStack() as ctx:
            wpool = ctx.enter_context(tc.tile_pool(name="w", bufs=1))
            xbpool = ctx.enter_context(tc.tile_pool(name="xb", bufs=2))
            hpool = ctx.enter_context(tc.tile_pool(name="h", bufs=34))
            opool = ctx.enter_context(tc.tile_pool(name="o", bufs=2))
            ps1p = ctx.enter_context(
                tc.tile_pool(name="ps1", bufs=4, space="PSUM"))
            ps2p = ctx.enter_context(
                tc.tile_pool(name="ps2", bufs=4, space="PSUM"))

            # DMA issue order gates the matmul waits (shared DMA semaphore,
            # order-based thresholds): block-0 activations first, then w1 in
            # H-quarters so mm1 h-tile 0 starts after ~2.5MB, then the rest.
            xnT_r = xnT.rearrange("(k p) t -> p k t", p=128)
            xb0 = xbpool.tile([128, KT, blocks[0]], BF16, tag="xb", name="xb_0")
            nc.sync.dma_start(xb0[:], xnT_r[:, :, 0:blocks[0]])
            w1_r = w1.rearrange("(k p) h -> p k h", p=128)
            w1_sb = wpool.tile([128, KT, H], BF16)
            nc.sync.dma_start(w1_sb[:, :, 0:H // 16], w1_r[:, :, 0:H // 16])
            b1_sb = wpool.tile([128, HT], F32)
            nc.sync.dma_start(b1_sb[:], b1r[:])
            cw_sb = wpool.tile([128, CR], F32)
            nc.sync.dma_start(cw_sb[:], cwr[:])
            nc.sync.dma_start(w1_sb[:, :, H // 16:H // 8],
                              w1_r[:, :, H // 16:H // 8])
            for q in range(1, 8):
                qsl = bass.ts(q, H // 8)
                nc.sync.dma_start(w1_sb[:, :, qsl], w1_r[:, :, qsl])
            w2_r = w2.rearrange("(k p) d -> p k d", p=128)
            w2_sb = wpool.tile([128, HT, D], BF16)
            nc.sync.dma_start(w2_sb[:, 0:HT // 2, :], w2_r[:, 0:HT // 2, :])
            nc.sync.dma_start(w2_sb[:, HT // 2:HT, :], w2_r[:, HT // 2:HT, :])

            tok0 = 0
            for b, blk in enumerate(blocks):
                if b == 0:
                    xb = xb0
                else:
                    xb = xbpool.tile([128, KT, blk], BF16, tag="xb",
                                     name=f"xb_{b}")
                    nc.sync.dma_start(xb[:], xnT_r[:, :, tok0:tok0 + blk])
                # mm1: h^T[ht] = gelu(w1_ht.T @ xn^T + b1)
                hts = []
                for ht in range(HT):
                    ps = ps1p.tile([128, blk], F32, tag="ps1",
                                   name=f"ps1_{b}_{ht}")
                    for k in range(KT):
                        nc.tensor.matmul(
                            ps[:], w1_sb[:, k, ht * 128:(ht + 1) * 128],
                            xb[:, k, :], start=(k == 0), stop=(k == KT - 1))
                    htile = hpool.tile([128, blk], BF16, tag="ht",
                                       name=f"ht_{b}_{ht}")
                    nc.scalar.activation(htile[:], ps[:], AF.Gelu,
                                         bias=b1_sb[:, ht:ht + 1])
                    hts.append(htile)
                # mm2: y[tok,:] = cw * (h^T.T @ w2); store in 256-tok chunks
                S = (blk + 127) // 128
                gstep = 1 if b == len(blocks) - 1 else 2
                for g in range(0, S, gstep):
                    gs = min(gstep, S - g)
                    ob = opool.tile([128, gs, D], F32, tag="ob",
                                    name=f"ob_{b}_{g}")
                    gfull = True
                    for j in range(gs):
                        ts_ = g + j
                        psz = min(128, blk - ts_ * 128)
                        gfull = gfull and psz == 128
                        tok_sl = bass.ds(ts_ * 128, psz)
                        ps2 = [ps2p.tile([128, 512], F32, tag="ps2",
                                         name=f"ps2_{b}_{ts_}_{i}")
                               for i in range(D // 512)]
                        for kh in range(HT):
                            for dc in range(D // 512):
                                nc.tensor.matmul(
                                    ps2[dc][:psz, :], hts[kh][:, tok_sl],
                                    w2_sb[:, kh, dc * 512:(dc + 1) * 512],
                                    start=(kh == 0), stop=(kh == HT - 1))
                        tok_i = tok0 // 128 + ts_
                        for dc in range(D // 512):
                            nc.vector.tensor_scalar_mul(
                                ob[:psz, j, dc * 512:(dc + 1) * 512],
                                ps2[dc][:psz, :], cw_sb[:psz, tok_i:tok_i + 1])
                    if gfull:
                        nc.sync.dma_start(
                            y_o[tok0 + g * 128:tok0 + (g + gs) * 128, :]
                            .rearrange("(s p) d -> p s d", p=128),
                            ob[:])
                    else:
                        psz = blk - g * 128
                        nc.sync.dma_start(
                            y_o[bass.ds(tok0 + g * 128, psz), :],
                            ob[:psz, 0, :])
                tok0 += blk

    nc.compile()
    return nc


# ---------------------------------------------------------------- host
def kernel(x, gate_w, w1, b1, w2, b2, gamma, beta):
    x = np.asarray(x, dtype=np.float32)
    gate_w = np.asarray(gate_w, dtype=np.float32)
    w1 = np.asarray(w1, dtype=np.float32)
    b1 = np.asarray(b1, dtype=np.float32)
    w2 = np.asarray(w2, dtype=np.float32)
    b2 = np.asarray(b2, dtype=np.float32)
    gamma = np.asarray(gamma, dtype=np.float32)
    beta = np.asarray(beta, dtype=np.float32)

    xt = np.ascontiguousarray(x.reshape(T, D))

    # ---- phase 1: LN + router ----
    affine = not (np.all(gamma == 1.0) and np.all(beta == 0.0))
    key1 = ("p1", affine)
    if key1 not in _cache:
        _cache[key1] = build_phase1(affine)
    nc1 = _cache[key1]
    in_maps1 = []
    for c in range(N_CORES):
        sl = xt[c * TC:(c + 1) * TC]
        m = {"xr": np.ascontiguousarray(sl),
             "xT": np.ascontiguousarray(sl.T),
             "gate_w": gate_w}
        if affine:
            m["gamma_r"] = np.ascontiguousarray(gamma.reshape(KT, 128).T)
            m["beta_r"] = np.ascontiguousarray(beta.reshape(KT, 128).T)
            m["gb"] = np.ascontiguousarray(np.broadcast_to(gamma, (128, D)))
            m["bb"] = np.ascontiguousarray(np.broadcast_to(beta, (128, D)))
        in_maps1.append(m)
    res1 = run_bass_kernel_spmd(nc1, in_maps1, list(range(N_CORES)))
    LAST_RESULTS["p1"] = res1
    xn_full = np.concatenate([res1.results[c]["xn"] for c in range(N_CORES)],
                             axis=0)             # [T, D] bf16 rows
    cw_full = np.concatenate([res1.results[c]["cw"] for c in range(N_CORES)],
                             axis=0)             # [T, E] f32

    # ---- host dispatch: gather tokens by expert ----
    idxs = [np.nonzero(cw_full[:, e] != 0.0)[0] for e in range(E)]
    counts = [len(ix) for ix in idxs]
    C = max(128, ((max(counts) + 127) // 128) * 128)
    CR = C // 128

    if ("p2", C) not in _cache:
        _cache[("p2", C)] = build_phase2(C)
    nc2 = _cache[("p2", C)]

    w1_b = w1.astype(ml_dtypes.bfloat16)
    w2_b = w2.astype(ml_dtypes.bfloat16)
    in_maps2 = []
    for e in range(E):
        ix = idxs[e]
        xnT_e = np.zeros((D, C), dtype=ml_dtypes.bfloat16)
        xnT_e[:, :counts[e]] = xn_full[ix].T
        cw_e = np.zeros((CR * 128,), dtype=np.float32)
        cw_e[:counts[e]] = cw_full[ix, e]
        in_maps2.append({
            "xnT": xnT_e,
            "w1": np.ascontiguousarray(w1_b[e]),
            "w2": np.ascontiguousarray(w2_b[e]),
            "b1r": np.ascontiguousarray(b1[e].reshape(HT, 128).T),
            "cwr": np.ascontiguousarray(cw_e.reshape(CR, 128).T),
        })
    res2 = run_bass_kernel_spmd(nc2, in_maps2, list(range(N_CORES)))
    LAST_RESULTS["p2"] = res2

    # ---- host combine: scatter-add + residual (+ per-expert b2, zero here)
    out = xt.copy()
    b2_any = bool(np.any(b2))
    for e in range(E):
        contrib = res2.results[e]["y"][:counts[e]]
        if b2_any:
            contrib = contrib + cw_full[idxs[e], e][:, None] * b2[e][None, :]
        out[idxs[e]] += contrib
    return out.reshape(B, L, D)



# revision 8
# speedup vs baseline: 1.2686x; 1.2686x over previous
"""CityExpertMoE Trainium2 kernel.

Two-phase, 8-core design:
  Phase 1 (data-parallel over tokens): LayerNorm + router logits + top-2
  combine weights, computed in transposed [D, tok] layout.
  Host: gather tokens by expert id ("all-to-all dispatch").
  Phase 2 (expert-parallel): core e runs expert e's FFN
  (1024 -> 4096 GELU -> 1024, bf16 matmuls, fp32 accumulate) on its
  gathered tokens, scales rows by the combine weight.
  Host: scatter-add partial outputs + residual + b2 (top-2 weights sum to 1).
"""

import sys
import types

import numpy as np
import ml_dtypes

# If BASS_TRACE is set but the axon NTFF hook shim is absent, bass_utils
# would fail importing antenv.axon_hooks; register a no-op fallback.
try:
    import antenv.axon_hooks  # noqa: F401
except ImportError:
    _m = types.ModuleType("antenv.axon_hooks")
    _m._hook = None
    _m.set_axon_ntff_profile_hook = lambda h: setattr(_m, "_hook", h)
    _m.get_axon_ntff_profile_hook = lambda: _m._hook
    sys.modules["antenv.axon_hooks"] = _m
    try:
        import antenv
        antenv.axon_hooks = _m
    except ImportError:
        pass

import concourse.bass as bass
import concourse.mybir as mybir
import concourse.tile as tile
from concourse import bacc
from concourse import masks
from concourse.bass_utils import run_bass_kernel_spmd

F32 = mybir.dt.float32
BF16 = mybir.dt.bfloat16
FP8 = mybir.dt.float8e4
DR = mybir.MatmulPerfMode.DoubleRow
AF = mybir.ActivationFunctionType
ALU = mybir.AluOpType
W2_SCALE = 16.0         # w2 pre-scaled into fp8 range; cw carries 1/16

B, L, D, H, E, TOP_K = 4, 2048, 1024, 4096, 8, 2
T = B * L               # 8192 tokens total
N_CORES = 8
TC = T // N_CORES       # 1024 tokens per core in phase 1
KT = D // 128           # 8 k-tiles over D
HT = H // 128           # 32 k-tiles over H
LN_EPS = 1e-5
BLK = 512               # phase-2 token block

_cache: dict = {}
LAST_RESULTS: dict = {}


# ---------------------------------------------------------------- phase 1
def build_phase1(affine: bool):
    """LayerNorm + router top-2. x comes in twice (rows and transposed).

    Pass 1 (per 128-token tile): bn_stats fused mean/var, rsqrt via
    Sqrt+fast-reciprocal, one fused normalize op writing bf16 directly.
    Router: logits = r*(x @ gwg) - r*mu*colsum(gwg) + beta @ gate_w
    (exact), computed from the raw x^T upload; per-token terms applied in
    [token, E] layout where r and mu*r are per-partition scalars.
    Pass 2: batched top-2 renormalized weights over all tiles at once.
    """
    nc = bacc.Bacc("TRN2", target_bir_lowering=False, debug=False,
                   num_devices=N_CORES)
    xr_d = nc.dram_tensor("xr", [TC, D], F32, kind="ExternalInput").ap()
    xT_d = nc.dram_tensor("xT", [D, TC], F32, kind="ExternalInput").ap()
    gate_w = nc.dram_tensor("gate_w", [D, E], F32, kind="ExternalInput").ap()
    if affine:
        gr_d = nc.dram_tensor("gamma_r", [128, KT], F32, kind="ExternalInput").ap()
        br_d = nc.dram_tensor("beta_r", [128, KT], F32, kind="ExternalInput").ap()
        gb_d = nc.dram_tensor("gb", [128, D], F32, kind="ExternalInput").ap()
        bb_d = nc.dram_tensor("bb", [128, D], F32, kind="ExternalInput").ap()
    xn_o = nc.dram_tensor("xn", [TC, D], BF16, kind="ExternalOutput").ap()
    cw_o = nc.dram_tensor("cw", [TC, E], F32, kind="ExternalOutput").ap()

    NTT = TC // 128      # 128-token tiles
    NCH = TC // 512

    with tile.TileContext(nc) as tc:
        import contextlib
        with contextlib.ExitStack() as ctx:
            const = ctx.enter_context(tc.tile_pool(name="const", bufs=1))
            xin = ctx.enter_context(tc.tile_pool(name="xin", bufs=3))
            xnp = ctx.enter_context(tc.tile_pool(name="xnp", bufs=2))
            big = ctx.enter_context(tc.tile_pool(name="big", bufs=1))
            work = ctx.enter_context(tc.tile_pool(name="work", bufs=4))
            pers = ctx.enter_context(tc.tile_pool(name="pers", bufs=1))
            ps_r = ctx.enter_context(
                tc.tile_pool(name="ps_r", bufs=3, space="PSUM"))
            ps_l = ctx.enter_context(
                tc.tile_pool(name="ps_l", bufs=3, space="PSUM"))

            gw_sb = const.tile([128, KT, E], F32)
            nc.sync.dma_start(gw_sb[:], gate_w.rearrange("(k p) e -> p k e", p=128))
            epst = const.tile([128, 1], F32)
            nc.vector.memset(epst[:], LN_EPS)
            zerot = const.tile([128, 1], F32)
            nc.vector.memset(zerot[:], 0.0)
            ident8 = const.tile([8, 8], F32)
            masks.make_identity(nc, ident8[:])
            ones_col = const.tile([128, 1], F32)
            nc.vector.memset(ones_col[:], 1.0)
            ones_row = const.tile([1, 128], F32)
            nc.vector.memset(ones_row[:], 1.0)
            if affine:
                g_r = const.tile([128, KT], F32)
                nc.sync.dma_start(g_r[:], gr_d[:])
                b_r = const.tile([128, KT], F32)
                nc.sync.dma_start(b_r[:], br_d[:])
                gb = const.tile([128, D], F32)
                nc.sync.dma_start(gb[:], gb_d[:])
                bb = const.tile([128, D], F32)
                nc.sync.dma_start(bb[:], bb_d[:])
                gwg = const.tile([128, KT, E], F32)
                for k in range(KT):
                    nc.vector.tensor_scalar(gwg[:, k, :], gw_sb[:, k, :],
                                            g_r[:, k:k + 1], None, ALU.mult)
            else:
                gwg = gw_sb

            # B = colsum(gwg) as [128, 1, E]-broadcastable row; C0 likewise
            ps_b = ps_l.tile([1, E], F32, tag="lg", name="ps_b")
            for k in range(KT):
                nc.tensor.matmul(ps_b[:], ones_col[:], gwg[:, k, :],
                                 start=(k == 0), stop=(k == KT - 1))
            b_row = work.tile([1, E], F32, tag="b_row")
            nc.vector.tensor_copy(b_row[:], ps_b[:])
            ps_bb = ps_l.tile([128, E], F32, tag="lg", name="ps_bb")
            nc.tensor.matmul(ps_bb[:], ones_row[:], b_row[:],
                             start=True, stop=True)
            B_b = const.tile([128, E], F32)
            nc.vector.tensor_copy(B_b[:], ps_bb[:])
            if affine:
                ps_c = ps_l.tile([1, E], F32, tag="lg", name="ps_c")
                for k in range(KT):
                    bgw = work.tile([128, E], F32, tag="bgw")
                    nc.vector.tensor_scalar(bgw[:], gw_sb[:, k, :],
                                            b_r[:, k:k + 1], None, ALU.mult)
                    nc.tensor.matmul(ps_c[:], ones_col[:], bgw[:],
                                     start=(k == 0), stop=(k == KT - 1))
                c_row = work.tile([1, E], F32, tag="c_row")
                nc.vector.tensor_copy(c_row[:], ps_c[:])
                ps_cb = ps_l.tile([128, E], F32, tag="lg", name="ps_cb")
                nc.tensor.matmul(ps_cb[:], ones_row[:], c_row[:],
                                 start=True, stop=True)
                C0_b = const.tile([128, E], F32)
                nc.vector.tensor_copy(C0_b[:], ps_cb[:])

            # interleave rows (LN) and x^T token-chunks (router) so both
            # pipelines chase the single saturated DMA stream
            xT_sb = big.tile([128, KT, TC], F32)
            xT_r = xT_d.rearrange("(k p) t -> p k t", p=128)
            xr_tiles = [xin.tile([128, D], F32, tag="xr", name=f"xr_{t}",
                                 bufs=NTT) for t in range(NTT)]
            for half in range(2):
                for t in range(half * (NTT // 2), (half + 1) * (NTT // 2)):
                    nc.sync.dma_start(xr_tiles[t][:], xr_d[bass.ts(t, 128), :])
                csl = bass.ts(half, TC // 2)
                nc.sync.dma_start(xT_sb[:, :, csl], xT_r[:, :, csl])
            A_row = big.tile([8, TC], F32)
            for ch in range(NCH):
                ps = ps_r.tile([8, 512], F32, tag="A", name=f"A_{ch}")
                for k in range(KT):
                    nc.tensor.matmul(ps[:], gwg[:, k, :],
                                     xT_sb[:, k, bass.ts(ch, 512)],
                                     start=(k == 0), stop=(k == KT - 1))
                nc.vector.tensor_copy(A_row[:, bass.ts(ch, 512)], ps[:])

            # ---- pass 1: LN per tile ----
            xnb_all = big.tile([128, NTT, D], BF16)
            r_all = pers.tile([128, NTT], F32)
            mrn_all = pers.tile([128, NTT], F32)
            t1_all = pers.tile([128, NTT, E], F32)
            for t in range(NTT):
                tsl = bass.ts(t, 128)
                xr = xr_tiles[t]
                bst = work.tile([128, 2, 6], F32, tag="bst")
                for g in range(2):
                    nc.vector.bn_stats(bst[:, g, :], xr[:, bass.ts(g, 512)])
                mv = work.tile([128, 2], F32, tag="mv")
                nc.vector.bn_aggr(mv[:], bst[:])
                std = work.tile([128, 1], F32, tag="std")
                nc.scalar.activation(std[:], mv[:, 1:2], AF.Sqrt, bias=epst[:])
                scr = work.tile([128, 1], F32, tag="scr")
                nc.vector.reciprocal_approx_accurate(r_all[:, t:t + 1], std[:],
                                                     scr[:])
                nc.vector.tensor_scalar(mrn_all[:, t:t + 1], mv[:, 0:1],
                                        r_all[:, t:t + 1], -1.0,
                                        ALU.mult, ALU.mult)
                if affine:
                    xn = xnp.tile([128, D], F32, tag="xn")
                    nc.vector.tensor_scalar(xn[:], xr[:], mv[:, 0:1],
                                            r_all[:, t:t + 1],
                                            ALU.subtract, ALU.mult)
                    nc.vector.tensor_mul(xn[:], xn[:], gb[:])
                    nc.vector.tensor_add(xnb_all[:, t, :], xn[:], bb[:])
                else:
                    nc.scalar.activation(xnb_all[:, t, :], xr[:], AF.Identity,
                                         bias=mrn_all[:, t:t + 1],
                                         scale=r_all[:, t:t + 1])
                # per-tile slice of the router correction: t1 = r * A_t
                At_ps = ps_l.tile([128, E], F32, tag="lg", name=f"At_{t}")
                nc.tensor.transpose(At_ps[:], A_row[:, tsl], ident8[:])
                nc.vector.tensor_scalar(t1_all[:, t, :], At_ps[:],
                                        r_all[:, t:t + 1], None, ALU.mult)

            xn_r = xn_o.rearrange("(t p) d -> p t d", p=128)
            hN = NTT // 2
            nc.sync.dma_start(xn_r[:, 0:hN, :], xnb_all[:, 0:hN, :])
            nc.sync.dma_start(xn_r[:, hN:NTT, :], xnb_all[:, hN:NTT, :])

            # ---- pass 2: batched top-2 over [128, NTT, E] ----
            def bc_t(ap_2d):     # [128, NTT] -> [128, NTT, E] (0-step E)
                return ap_2d.to_broadcast((128, NTT, E))

            def bc_e(ap_2d):     # [128, E] -> [128, NTT, E] (0-step NTT)
                return ap_2d.rearrange("p (t e) -> p t e",
                                       t=1).to_broadcast((128, NTT, E))

            lg = pers.tile([128, NTT, E], F32)
            nc.vector.tensor_tensor(lg[:], bc_t(mrn_all[:]), bc_e(B_b[:]),
                                    ALU.mult)
            nc.vector.tensor_add(lg[:], lg[:], t1_all[:])
            if affine:
                nc.vector.tensor_tensor(lg[:], lg[:], bc_e(C0_b[:]), ALU.add)
            m1 = pers.tile([128, NTT], F32)
            nc.vector.reduce_max(m1[:], lg[:], axis=mybir.AxisListType.X)
            eq = pers.tile([128, NTT, E], F32)
            nc.vector.tensor_tensor(eq[:], lg[:], bc_t(m1[:]), ALU.is_equal)
            masked = pers.tile([128, NTT, E], F32)
            nc.vector.scalar_tensor_tensor(masked[:], eq[:], -1e30, lg[:],
                                           ALU.mult, ALU.add)
            m2 = pers.tile([128, NTT], F32)
            nc.vector.reduce_max(m2[:], masked[:], axis=mybir.AxisListType.X)
            ge = pers.tile([128, NTT, E], F32)
            nc.vector.tensor_tensor(ge[:], lg[:], bc_t(m2[:]), ALU.is_ge)
            sub = pers.tile([128, NTT, E], F32)
            nc.vector.tensor_tensor(sub[:], lg[:], bc_t(m1[:]), ALU.subtract)
            exps = pers.tile([128, NTT, E], F32)
            nc.scalar.activation(exps[:], sub[:], AF.Exp, bias=zerot[:])
            cw_raw = pers.tile([128, NTT, E], F32)
            nc.vector.tensor_mul(cw_raw[:], exps[:], ge[:])
            den = pers.tile([128, NTT], F32)
            nc.vector.reduce_sum(den[:], cw_raw[:], axis=mybir.AxisListType.X)
            rec = pers.tile([128, NTT], F32)
            scr2 = pers.tile([128, NTT], F32)
            nc.vector.reciprocal_approx_accurate(rec[:], den[:], scr2[:])
            cw_all = pers.tile([128, NTT, E], F32)
            nc.vector.tensor_tensor(cw_all[:], cw_raw[:], bc_t(rec[:]),
                                    ALU.mult)
            nc.sync.dma_start(cw_o.rearrange("(t p) e -> p t e", p=128),
                              cw_all[:])

    nc.compile()
    return nc


# ---------------------------------------------------------------- phase 2
def build_phase2(C: int):
    """Expert FFN on C gathered (padded) tokens; one expert per core.

    Blocks of 512 tokens; only the final block may be ragged (any size)."""
    blocks = [512] * (C // 512)
    r = C % 512
    if r:
        if r < 256 and blocks:
            blocks.pop()
            total = 512 + r
            first = ((total + 1) // 2 + 127) // 128 * 128
            blocks.extend([first, total - first])
        else:
            blocks.append(r)
    nc = bacc.Bacc("TRN2", target_bir_lowering=False, debug=False,
                   num_devices=N_CORES)
    xnT = nc.dram_tensor("xnT", [D, C], BF16, kind="ExternalInput").ap()
    w1 = nc.dram_tensor("w1", [D, H], BF16, kind="ExternalInput").ap()
    w2 = nc.dram_tensor("w2", [H, D], FP8, kind="ExternalInput").ap()
    b1r = nc.dram_tensor("b1r", [128, HT], F32, kind="ExternalInput").ap()
    CR = (C + 127) // 128
    cwr = nc.dram_tensor("cwr", [128, CR], F32, kind="ExternalInput").ap()
    y_o = nc.dram_tensor("y", [C, D], BF16, kind="ExternalOutput").ap()

    with tile.TileContext(nc) as tc:
        import contextlib
        with contextlib.ExitStack() as ctx:
            wpool = ctx.enter_context(tc.tile_pool(name="w", bufs=1))
            xbpool = ctx.enter_context(tc.tile_pool(name="xb", bufs=2))
            hpool = ctx.enter_context(tc.tile_pool(name="h", bufs=2))
            opool = ctx.enter_context(tc.tile_pool(name="o", bufs=2))
            ps1p = ctx.enter_context(
                tc.tile_pool(name="ps1", bufs=4, space="PSUM"))
            ps2p = ctx.enter_context(
                tc.tile_pool(name="ps2", bufs=4, space="PSUM"))

            # DMA issue order gates the matmul waits (shared DMA semaphore,
            # order-based thresholds): block-0 activations first, then w1 in
            # H-quarters so mm1 h-tile 0 starts after ~2.5MB, then the rest.
            xnT_r = xnT.rearrange("(k p) t -> p k t", p=128)
            xb0 = xbpool.tile([128, KT, blocks[0]], BF16, tag="xb", name="xb_0")
            nc.sync.dma_start(xb0[:], xnT_r[:, :, 0:blocks[0]])
            w1_r = w1.rearrange("(k p) h -> p k h", p=128)
            w1_sb = wpool.tile([128, KT, H], BF16)
            nc.sync.dma_start(w1_sb[:, :, 0:H // 16], w1_r[:, :, 0:H // 16])
            b1_sb = wpool.tile([128, HT], F32)
            nc.sync.dma_start(b1_sb[:], b1r[:])
            cw_sb = wpool.tile([128, CR], F32)
            nc.sync.dma_start(cw_sb[:], cwr[:])
            nc.sync.dma_start(w1_sb[:, :, H // 16:H // 8],
                              w1_r[:, :, H // 16:H // 8])
            for q in range(1, 8):
                qsl = bass.ts(q, H // 8)
                nc.sync.dma_start(w1_sb[:, :, qsl], w1_r[:, :, qsl])
            w2_r = w2.rearrange("(k p) d -> p k d", p=128)
            w2_sb = wpool.tile([128, HT, D], FP8)
            nc.sync.dma_start(w2_sb[:, 0:HT // 2, :], w2_r[:, 0:HT // 2, :])
            nc.sync.dma_start(w2_sb[:, HT // 2:HT, :], w2_r[:, HT // 2:HT, :])

            tok0 = 0
            for b, blk in enumerate(blocks):
                if b == 0:
                    xb = xb0
                else:
                    xb = xbpool.tile([128, KT, blk], BF16, tag="xb",
                                     name=f"xb_{b}")
                    nc.sync.dma_start(xb[:], xnT_r[:, :, tok0:tok0 + blk])
                # mm1: h^T[ht] = gelu(w1_ht.T @ xn^T + b1), fp8 h
                h_all = hpool.tile([128, HT, blk], FP8, tag="ht",
                                   name=f"h_{b}")
                for ht in range(HT):
                    ps = ps1p.tile([128, blk], F32, tag="ps1",
                                   name=f"ps1_{b}_{ht}")
                    for k in range(KT):
                        nc.tensor.matmul(
                            ps[:], w1_sb[:, k, ht * 128:(ht + 1) * 128],
                            xb[:, k, :], start=(k == 0), stop=(k == KT - 1))
                    nc.scalar.activation(h_all[:, ht, :], ps[:], AF.Gelu,
                                         bias=b1_sb[:, ht:ht + 1])
                # mm2 (fp8 DoubleRow): y[tok,:] = cw/16 * (h^T.T @ 16*w2)
                S = (blk + 127) // 128
                gstep = 1 if b == len(blocks) - 1 else 2
                for g in range(0, S, gstep):
                    gs = min(gstep, S - g)
                    ob = opool.tile([128, gs, D], BF16, tag="ob",
                                    name=f"ob_{b}_{g}")
                    gfull = True
                    for j in range(gs):
                        ts_ = g + j
                        psz = min(128, blk - ts_ * 128)
                        gfull = gfull and psz == 128
                        tok_sl = bass.ds(ts_ * 128, psz)
                        ps2 = [ps2p.tile([128, 512], F32, tag="ps2",
                                         name=f"ps2_{b}_{ts_}_{i}")
                               for i in range(D // 512)]
                        for kh in range(0, HT, 2):
                            for dc in range(D // 512):
                                nc.tensor.matmul(
                                    ps2[dc][:psz, :],
                                    h_all[:, kh:kh + 2, tok_sl],
                                    w2_sb[:, kh:kh + 2,
                                          dc * 512:(dc + 1) * 512],
                                    start=(kh == 0), stop=(kh == HT - 2),
                                    perf_mode=DR)
                        tok_i = tok0 // 128 + ts_
                        for dc in range(D // 512):
                            nc.vector.tensor_scalar_mul(
                                ob[:psz, j, dc * 512:(dc + 1) * 512],
                                ps2[dc][:psz, :], cw_sb[:psz, tok_i:tok_i + 1])
                    if gfull:
                        nc.sync.dma_start(
                            y_o[tok0 + g * 128:tok0 + (g + gs) * 128, :]
                            .rearrange("(s p) d -> p s d", p=128),
                            ob[:])
                    else:
                        psz = blk - g * 128
                        nc.sync.dma_start(
                            y_o[bass.ds(tok0 + g * 128, psz), :],
                            ob[:psz, 0, :])
                tok0 += blk

    nc.compile()
    return nc


# ---------------------------------------------------------------- host
def kernel(x, gate_w, w1, b1, w2, b2, gamma, beta):
    x = np.asarray(x, dtype=np.float32)
    gate_w = np.asarray(gate_w, dtype=np.float32)
    w1 = np.asarray(w1, dtype=np.float32)
    b1 = np.asarray(b1, dtype=np.float32)
    w2 = np.asarray(w2, dtype=np.float32)
    b2 = np.asarray(b2, dtype=np.float32)
    gamma = np.asarray(gamma, dtype=np.float32)
    beta = np.asarray(beta, dtype=np.float32)

    xt = np.ascontiguousarray(x.reshape(T, D))

    # ---- phase 1: LN + router ----
    affine = not (np.all(gamma == 1.0) and np.all(beta == 0.0))
    key1 = ("p1", affine)
    if key1 not in _cache:
        _cache[key1] = build_phase1(affine)
    nc1 = _cache[key1]
    in_maps1 = []
    for c in range(N_CORES):
        sl = xt[c * TC:(c + 1) * TC]
        m = {"xr": np.ascontiguousarray(sl),
             "xT": np.ascontiguousarray(sl.T),
             "gate_w": gate_w}
        if affine:
            m["gamma_r"] = np.ascontiguousarray(gamma.reshape(KT, 128).T)
            m["beta_r"] = np.ascontiguousarray(beta.reshape(KT, 128).T)
            m["gb"] = np.ascontiguousarray(np.broadcast_to(gamma, (128, D)))
            m["bb"] = np.ascontiguousarray(np.broadcast_to(beta, (128, D)))
        in_maps1.append(m)
    res1 = run_bass_kernel_spmd(nc1, in_maps1, list(range(N_CORES)))
    LAST_RESULTS["p1"] = res1
    xn_full = np.concatenate([res1.results[c]["xn"] for c in range(N_CORES)],
                             axis=0)             # [T, D] bf16 rows
    cw_full = np.concatenate([res1.results[c]["cw"] for c in range(N_CORES)],
                             axis=0)             # [T, E] f32

    # ---- host dispatch: gather tokens by expert ----
    idxs = [np.nonzero(cw_full[:, e] != 0.0)[0] for e in range(E)]
    counts = [len(ix) for ix in idxs]
    C = max(128, ((max(counts) + 127) // 128) * 128)
    CR = C // 128

    if ("p2", C) not in _cache:
        _cache[("p2", C)] = build_phase2(C)
    nc2 = _cache[("p2", C)]

    w1_b = w1.astype(ml_dtypes.bfloat16)
    w2_8 = np.clip(w2 * W2_SCALE, -240, 240).astype(ml_dtypes.float8_e4m3fn)
    in_maps2 = []
    for e in range(E):
        ix = idxs[e]
        xnT_e = np.zeros((D, C), dtype=ml_dtypes.bfloat16)
        xnT_e[:, :counts[e]] = xn_full[ix].T
        cw_e = np.zeros((CR * 128,), dtype=np.float32)
        cw_e[:counts[e]] = cw_full[ix, e] / W2_SCALE
        in_maps2.append({
            "xnT": xnT_e,
            "w1": np.ascontiguousarray(w1_b[e]),
            "w2": np.ascontiguousarray(w2_8[e]),
            "b1r": np.ascontiguousarray(b1[e].reshape(HT, 128).T),
            "cwr": np.ascontiguousarray(cw_e.reshape(CR, 128).T),
        })
    res2 = run_bass_kernel_spmd(nc2, in_maps2, list(range(N_CORES)))
    LAST_RESULTS["p2"] = res2

    # ---- host combine: scatter-add + residual (+ per-expert b2, zero here)
    out = xt.copy()
    b2_any = bool(np.any(b2))
    for e in range(E):
        contrib = res2.results[e]["y"][:counts[e]].astype(np.float32)
        if b2_any:
            contrib = contrib + cw_full[idxs[e], e][:, None] * b2[e][None, :]
        out[idxs[e]] += contrib
    return out.reshape(B, L, D)



# revision 13
# speedup vs baseline: 1.2706x; 1.0016x over previous
"""CityExpertMoE Trainium2 kernel.

Two-phase, 8-core design:
  Phase 1 (data-parallel over tokens): LayerNorm + router logits + top-2
  combine weights, computed in transposed [D, tok] layout.
  Host: gather tokens by expert id ("all-to-all dispatch").
  Phase 2 (expert-parallel): core e runs expert e's FFN
  (1024 -> 4096 GELU -> 1024, bf16 matmuls, fp32 accumulate) on its
  gathered tokens, scales rows by the combine weight.
  Host: scatter-add partial outputs + residual + b2 (top-2 weights sum to 1).
"""

import sys
import types

import numpy as np
import ml_dtypes

# If BASS_TRACE is set but the axon NTFF hook shim is absent, bass_utils
# would fail importing antenv.axon_hooks; register a no-op fallback.
try:
    import antenv.axon_hooks  # noqa: F401
except ImportError:
    _m = types.ModuleType("antenv.axon_hooks")
    _m._hook = None
    _m.set_axon_ntff_profile_hook = lambda h: setattr(_m, "_hook", h)
    _m.get_axon_ntff_profile_hook = lambda: _m._hook
    sys.modules["antenv.axon_hooks"] = _m
    try:
        import antenv
        antenv.axon_hooks = _m
    except ImportError:
        pass

import concourse.bass as bass
import concourse.mybir as mybir
import concourse.tile as tile
from concourse import bacc
from concourse import masks
from concourse.bass_utils import run_bass_kernel_spmd

F32 = mybir.dt.float32
BF16 = mybir.dt.bfloat16
FP8 = mybir.dt.float8e4
DR = mybir.MatmulPerfMode.DoubleRow
AF = mybir.ActivationFunctionType
ALU = mybir.AluOpType
W2_SCALE = 16.0         # w2 pre-scaled into fp8 range; cw carries 1/16

B, L, D, H, E, TOP_K = 4, 2048, 1024, 4096, 8, 2
T = B * L               # 8192 tokens total
N_CORES = 8
TC = T // N_CORES       # 1024 tokens per core in phase 1
KT = D // 128           # 8 k-tiles over D
HT = H // 128           # 32 k-tiles over H
LN_EPS = 1e-5
BLK = 512               # phase-2 token block

_cache: dict = {}
LAST_RESULTS: dict = {}


# ---------------------------------------------------------------- phase 1
def build_phase1(affine: bool):
    """LayerNorm + router top-2. x comes in twice (rows and transposed).

    Pass 1 (per 128-token tile): bn_stats fused mean/var, rsqrt via
    Sqrt+fast-reciprocal, one fused normalize op writing bf16 directly.
    Router: logits = r*(x @ gwg) - r*mu*colsum(gwg) + beta @ gate_w
    (exact), computed from the raw x^T upload; per-token terms applied in
    [token, E] layout where r and mu*r are per-partition scalars.
    Pass 2: batched top-2 renormalized weights over all tiles at once.
    """
    nc = bacc.Bacc("TRN2", target_bir_lowering=False, debug=False,
                   num_devices=N_CORES)
    xr_d = nc.dram_tensor("xr", [TC, D], BF16, kind="ExternalInput").ap()
    xT_d = nc.dram_tensor("xT", [D, TC], F32, kind="ExternalInput").ap()
    gate_w = nc.dram_tensor("gate_w", [D, E], F32, kind="ExternalInput").ap()
    if affine:
        gr_d = nc.dram_tensor("gamma_r", [128, KT], F32, kind="ExternalInput").ap()
        br_d = nc.dram_tensor("beta_r", [128, KT], F32, kind="ExternalInput").ap()
        gb_d = nc.dram_tensor("gb", [128, D], F32, kind="ExternalInput").ap()
        bb_d = nc.dram_tensor("bb", [128, D], F32, kind="ExternalInput").ap()
    xn_o = nc.dram_tensor("xn", [TC, D], BF16, kind="ExternalOutput").ap()
    cw_o = nc.dram_tensor("cw", [TC, E], F32, kind="ExternalOutput").ap()

    NTT = TC // 128      # 128-token tiles
    NCH = TC // 512

    with tile.TileContext(nc) as tc:
        import contextlib
        with contextlib.ExitStack() as ctx:
            const = ctx.enter_context(tc.tile_pool(name="const", bufs=1))
            xin = ctx.enter_context(tc.tile_pool(name="xin", bufs=3))
            xnp = ctx.enter_context(tc.tile_pool(name="xnp", bufs=2))
            big = ctx.enter_context(tc.tile_pool(name="big", bufs=1))
            work = ctx.enter_context(tc.tile_pool(name="work", bufs=4))
            pers = ctx.enter_context(tc.tile_pool(name="pers", bufs=1))
            ps_r = ctx.enter_context(
                tc.tile_pool(name="ps_r", bufs=3, space="PSUM"))
            ps_l = ctx.enter_context(
                tc.tile_pool(name="ps_l", bufs=3, space="PSUM"))
            ps_wp = ctx.enter_context(
                tc.tile_pool(name="ps_wp", bufs=1, space="PSUM"))

            # PE warmup during the xT upload (see phase 2)
            wz = const.tile([128, 512], BF16)
            nc.vector.memset(wz[:], 0.0)
            ps_w = ps_wp.tile([128, 512], F32, tag="warm", name="ps_warm")
            for i in range(30):
                nc.tensor.matmul(ps_w[:], wz[:, 0:128], wz[:],
                                 start=True, stop=True)

            gw_sb = const.tile([128, KT, E], F32)
            nc.sync.dma_start(gw_sb[:], gate_w.rearrange("(k p) e -> p k e", p=128))
            epst = const.tile([128, 1], F32)
            nc.vector.memset(epst[:], LN_EPS)
            zerot = const.tile([128, 1], F32)
            nc.vector.memset(zerot[:], 0.0)
            ident8 = const.tile([8, 8], F32)
            masks.make_identity(nc, ident8[:])
            ones_col = const.tile([128, 1], F32)
            nc.vector.memset(ones_col[:], 1.0)
            ones_row = const.tile([1, 128], F32)
            nc.vector.memset(ones_row[:], 1.0)
            if affine:
                g_r = const.tile([128, KT], F32)
                nc.sync.dma_start(g_r[:], gr_d[:])
                b_r = const.tile([128, KT], F32)
                nc.sync.dma_start(b_r[:], br_d[:])
                gb = const.tile([128, D], F32)
                nc.sync.dma_start(gb[:], gb_d[:])
                bb = const.tile([128, D], F32)
                nc.sync.dma_start(bb[:], bb_d[:])
                gwg = const.tile([128, KT, E], F32)
                for k in range(KT):
                    nc.vector.tensor_scalar(gwg[:, k, :], gw_sb[:, k, :],
                                            g_r[:, k:k + 1], None, ALU.mult)
            else:
                gwg = gw_sb

            # B = colsum(gwg) as [128, 1, E]-broadcastable row; C0 likewise
            ps_b = ps_l.tile([1, E], F32, tag="lg", name="ps_b")
            for k in range(KT):
                nc.tensor.matmul(ps_b[:], ones_col[:], gwg[:, k, :],
                                 start=(k == 0), stop=(k == KT - 1))
            b_row = work.tile([1, E], F32, tag="b_row")
            nc.vector.tensor_copy(b_row[:], ps_b[:])
            ps_bb = ps_l.tile([128, E], F32, tag="lg", name="ps_bb")
            nc.tensor.matmul(ps_bb[:], ones_row[:], b_row[:],
                             start=True, stop=True)
            B_b = const.tile([128, E], F32)
            nc.vector.tensor_copy(B_b[:], ps_bb[:])
            if affine:
                ps_c = ps_l.tile([1, E], F32, tag="lg", name="ps_c")
                for k in range(KT):
                    bgw = work.tile([128, E], F32, tag="bgw")
                    nc.vector.tensor_scalar(bgw[:], gw_sb[:, k, :],
                                            b_r[:, k:k + 1], None, ALU.mult)
                    nc.tensor.matmul(ps_c[:], ones_col[:], bgw[:],
                                     start=(k == 0), stop=(k == KT - 1))
                c_row = work.tile([1, E], F32, tag="c_row")
                nc.vector.tensor_copy(c_row[:], ps_c[:])
                ps_cb = ps_l.tile([128, E], F32, tag="lg", name="ps_cb")
                nc.tensor.matmul(ps_cb[:], ones_row[:], c_row[:],
                                 start=True, stop=True)
                C0_b = const.tile([128, E], F32)
                nc.vector.tensor_copy(C0_b[:], ps_cb[:])

            # interleave rows (LN) and x^T token-chunks (router) so both
            # pipelines chase the single saturated DMA stream
            xT_sb = big.tile([128, KT, TC], F32)
            xT_r = xT_d.rearrange("(k p) t -> p k t", p=128)
            xr_tiles = [xin.tile([128, D], BF16, tag="xr", name=f"xr_{t}",
                                 bufs=NTT) for t in range(NTT)]
            for half in range(2):
                for t in range(half * (NTT // 2), (half + 1) * (NTT // 2)):
                    nc.sync.dma_start(xr_tiles[t][:], xr_d[bass.ts(t, 128), :])
                csl = bass.ts(half, TC // 2)
                nc.sync.dma_start(xT_sb[:, :, csl], xT_r[:, :, csl])
            A_row = big.tile([8, TC], F32)
            for ch in range(NCH):
                ps = ps_r.tile([8, 512], F32, tag="A", name=f"A_{ch}")
                for k in range(KT):
                    nc.tensor.matmul(ps[:], gwg[:, k, :],
                                     xT_sb[:, k, bass.ts(ch, 512)],
                                     start=(k == 0), stop=(k == KT - 1))
                nc.vector.tensor_copy(A_row[:, bass.ts(ch, 512)], ps[:])

            # ---- pass 1: LN per tile ----
            xnb_all = big.tile([128, NTT, D], BF16)
            r_all = pers.tile([128, NTT], F32)
            mrn_all = pers.tile([128, NTT], F32)
            t1_all = pers.tile([128, NTT, E], F32)
            for t in range(NTT):
                tsl = bass.ts(t, 128)
                xr = xr_tiles[t]
                bst = work.tile([128, 2, 6], F32, tag="bst")
                for g in range(2):
                    nc.vector.bn_stats(bst[:, g, :], xr[:, bass.ts(g, 512)])
                mv = work.tile([128, 2], F32, tag="mv")
                nc.vector.bn_aggr(mv[:], bst[:])
                std = work.tile([128, 1], F32, tag="std")
                nc.scalar.activation(std[:], mv[:, 1:2], AF.Sqrt, bias=epst[:])
                scr = work.tile([128, 1], F32, tag="scr")
                nc.vector.reciprocal_approx_accurate(r_all[:, t:t + 1], std[:],
                                                     scr[:])
                nc.vector.tensor_scalar(mrn_all[:, t:t + 1], mv[:, 0:1],
                                        r_all[:, t:t + 1], -1.0,
                                        ALU.mult, ALU.mult)
                if affine:
                    xn = xnp.tile([128, D], F32, tag="xn")
                    nc.vector.tensor_scalar(xn[:], xr[:], mv[:, 0:1],
                                            r_all[:, t:t + 1],
                                            ALU.subtract, ALU.mult)
                    nc.vector.tensor_mul(xn[:], xn[:], gb[:])
                    nc.vector.tensor_add(xnb_all[:, t, :], xn[:], bb[:])
                else:
                    nc.scalar.activation(xnb_all[:, t, :], xr[:], AF.Identity,
                                         bias=mrn_all[:, t:t + 1],
                                         scale=r_all[:, t:t + 1])
                # per-tile slice of the router correction: t1 = r * A_t
                At_ps = ps_l.tile([128, E], F32, tag="lg", name=f"At_{t}")
                nc.tensor.transpose(At_ps[:], A_row[:, tsl], ident8[:])
                nc.vector.tensor_scalar(t1_all[:, t, :], At_ps[:],
                                        r_all[:, t:t + 1], None, ALU.mult)

            xn_r = xn_o.rearrange("(t p) d -> p t d", p=128)
            hN = NTT // 2
            nc.sync.dma_start(xn_r[:, 0:hN, :], xnb_all[:, 0:hN, :])
            nc.sync.dma_start(xn_r[:, hN:NTT, :], xnb_all[:, hN:NTT, :])

            # ---- pass 2: batched top-2 over [128, NTT, E] ----
            def bc_t(ap_2d):     # [128, NTT] -> [128, NTT, E] (0-step E)
                return ap_2d.to_broadcast((128, NTT, E))

            def bc_e(ap_2d):     # [128, E] -> [128, NTT, E] (0-step NTT)
                return ap_2d.rearrange("p (t e) -> p t e",
                                       t=1).to_broadcast((128, NTT, E))

            lg = pers.tile([128, NTT, E], F32)
            nc.vector.tensor_tensor(lg[:], bc_t(mrn_all[:]), bc_e(B_b[:]),
                                    ALU.mult)
            nc.vector.tensor_add(lg[:], lg[:], t1_all[:])
            if affine:
                nc.vector.tensor_tensor(lg[:], lg[:], bc_e(C0_b[:]), ALU.add)
            m1 = pers.tile([128, NTT], F32)
            nc.vector.reduce_max(m1[:], lg[:], axis=mybir.AxisListType.X)
            eq = pers.tile([128, NTT, E], F32)
            nc.vector.tensor_tensor(eq[:], lg[:], bc_t(m1[:]), ALU.is_equal)
            masked = pers.tile([128, NTT, E], F32)
            nc.vector.scalar_tensor_tensor(masked[:], eq[:], -1e30, lg[:],
                                           ALU.mult, ALU.add)
            m2 = pers.tile([128, NTT], F32)
            nc.vector.reduce_max(m2[:], masked[:], axis=mybir.AxisListType.X)
            ge = pers.tile([128, NTT, E], F32)
            nc.vector.tensor_tensor(ge[:], lg[:], bc_t(m2[:]), ALU.is_ge)
            sub = pers.tile([128, NTT, E], F32)
            nc.vector.tensor_tensor(sub[:], lg[:], bc_t(m1[:]), ALU.subtract)
            exps = pers.tile([128, NTT, E], F32)
            nc.scalar.activation(exps[:], sub[:], AF.Exp, bias=zerot[:])
            cw_raw = pers.tile([128, NTT, E], F32)
            nc.vector.tensor_mul(cw_raw[:], exps[:], ge[:])
            den = pers.tile([128, NTT], F32)
            nc.vector.reduce_sum(den[:], cw_raw[:], axis=mybir.AxisListType.X)
            rec = pers.tile([128, NTT], F32)
            scr2 = pers.tile([128, NTT], F32)
            nc.vector.reciprocal_approx_accurate(rec[:], den[:], scr2[:])
            cw_all = pers.tile([128, NTT, E], F32)
            nc.vector.tensor_tensor(cw_all[:], cw_raw[:], bc_t(rec[:]),
                                    ALU.mult)
            nc.sync.dma_start(cw_o.rearrange("(t p) e -> p t e", p=128),
                              cw_all[:])

    nc.compile()
    return nc


# ---------------------------------------------------------------- phase 2
def build_phase2(C: int):
    """Expert FFN on C gathered (padded) tokens; one expert per core.

    Blocks of 512 tokens; only the final block may be ragged (any size)."""
    blocks = [512] * (C // 512)
    r = C % 512
    if r:
        if r < 256 and blocks:
            blocks.pop()
            total = 512 + r
            first = ((total + 1) // 2 + 127) // 128 * 128
            blocks.extend([first, total - first])
        else:
            blocks.append(r)
    nc = bacc.Bacc("TRN2", target_bir_lowering=False, debug=False,
                   num_devices=N_CORES)
    xnT = nc.dram_tensor("xnT", [D, C], BF16, kind="ExternalInput").ap()
    w1 = nc.dram_tensor("w1", [D, H], BF16, kind="ExternalInput").ap()
    w2 = nc.dram_tensor("w2", [H, D], FP8, kind="ExternalInput").ap()
    b1r = nc.dram_tensor("b1r", [128, HT], F32, kind="ExternalInput").ap()
    CR = (C + 127) // 128
    cwr = nc.dram_tensor("cwr", [128, CR], F32, kind="ExternalInput").ap()
    y_o = nc.dram_tensor("y", [C, D], BF16, kind="ExternalOutput").ap()

    with tile.TileContext(nc) as tc:
        import contextlib
        with contextlib.ExitStack() as ctx:
            wpool = ctx.enter_context(tc.tile_pool(name="w", bufs=1))
            xbpool = ctx.enter_context(tc.tile_pool(name="xb", bufs=2))
            hpool = ctx.enter_context(tc.tile_pool(name="h", bufs=2))
            opool = ctx.enter_context(tc.tile_pool(name="o", bufs=2))
            ps1p = ctx.enter_context(
                tc.tile_pool(name="ps1", bufs=4, space="PSUM"))
            ps2p = ctx.enter_context(
                tc.tile_pool(name="ps2", bufs=4, space="PSUM"))

            # PE warmup: keep TensorE busy through the initial DMA wait so
            # HAM un-throttles (1.2 -> 2.4 GHz) before the real matmuls.
            wz = wpool.tile([128, 512], BF16)
            nc.vector.memset(wz[:], 0.0)
            ps_w = ps1p.tile([128, 512], F32, tag="ps1", name="ps_warm")
            for i in range(48):
                nc.tensor.matmul(ps_w[:], wz[:, 0:128], wz[:],
                                 start=True, stop=True)

            # DMA issue order gates the matmul waits (shared DMA semaphore,
            # order-based thresholds): block-0 activations first, then w1 in
            # H-quarters so mm1 h-tile 0 starts after ~2.5MB, then the rest.
            xnT_r = xnT.rearrange("(k p) t -> p k t", p=128)
            xb0 = xbpool.tile([128, KT, blocks[0]], BF16, tag="xb", name="xb_0")
            nc.sync.dma_start(xb0[:], xnT_r[:, :, 0:blocks[0]])
            w1_r = w1.rearrange("(k p) h -> p k h", p=128)
            w1_sb = wpool.tile([128, KT, H], BF16)
            nc.sync.dma_start(w1_sb[:, :, 0:H // 16], w1_r[:, :, 0:H // 16])
            b1_sb = wpool.tile([128, HT], F32)
            nc.sync.dma_start(b1_sb[:], b1r[:])
            cw_sb = wpool.tile([128, CR], F32)
            nc.sync.dma_start(cw_sb[:], cwr[:])
            nc.sync.dma_start(w1_sb[:, :, H // 16:H // 8],
                              w1_r[:, :, H // 16:H // 8])
            for q in range(1, 8):
                qsl = bass.ts(q, H // 8)
                nc.sync.dma_start(w1_sb[:, :, qsl], w1_r[:, :, qsl])
            w2_r = w2.rearrange("(k p) d -> p k d", p=128)
            w2_sb = wpool.tile([128, HT, D], FP8)
            nc.sync.dma_start(w2_sb[:, 0:HT // 2, :], w2_r[:, 0:HT // 2, :])
            nc.sync.dma_start(w2_sb[:, HT // 2:HT, :], w2_r[:, HT // 2:HT, :])

            tok0 = 0
            for b, blk in enumerate(blocks):
                if b == 0:
                    xb = xb0
                else:
                    xb = xbpool.tile([128, KT, blk], BF16, tag="xb",
                                     name=f"xb_{b}")
                    nc.sync.dma_start(xb[:], xnT_r[:, :, tok0:tok0 + blk])
                # mm1: h^T[ht] = gelu(w1_ht.T @ xn^T + b1), fp8 h
                h_all = hpool.tile([128, HT, blk], FP8, tag="ht",
                                   name=f"h_{b}")
                for ht in range(HT):
                    ps = ps1p.tile([128, blk], F32, tag="ps1",
                                   name=f"ps1_{b}_{ht}")
                    for k in range(KT):
                        nc.tensor.matmul(
                            ps[:], w1_sb[:, k, ht * 128:(ht + 1) * 128],
                            xb[:, k, :], start=(k == 0), stop=(k == KT - 1))
                    nc.scalar.activation(h_all[:, ht, :], ps[:], AF.Gelu,
                                         bias=b1_sb[:, ht:ht + 1])
                # mm2 (fp8 DoubleRow): y[tok,:] = cw/16 * (h^T.T @ 16*w2)
                S = (blk + 127) // 128
                gstep = 1 if b == len(blocks) - 1 else 2
                for g in range(0, S, gstep):
                    gs = min(gstep, S - g)
                    ob = opool.tile([128, gs, D], BF16, tag="ob",
                                    name=f"ob_{b}_{g}")
                    gfull = True
                    for j in range(gs):
                        ts_ = g + j
                        psz = min(128, blk - ts_ * 128)
                        gfull = gfull and psz == 128
                        tok_sl = bass.ds(ts_ * 128, psz)
                        ps2 = [ps2p.tile([128, 512], F32, tag="ps2",
                                         name=f"ps2_{b}_{ts_}_{i}")
                               for i in range(D // 512)]
                        for kh in range(0, HT, 2):
                            for dc in range(D // 512):
                                nc.tensor.matmul(
                                    ps2[dc][:psz, :],
                                    h_all[:, kh:kh + 2, tok_sl],
                                    w2_sb[:, kh:kh + 2,
                                          dc * 512:(dc + 1) * 512],
                                    start=(kh == 0), stop=(kh == HT - 2),
                                    perf_mode=DR)
                        tok_i = tok0 // 128 + ts_
                        for dc in range(D // 512):
                            nc.vector.tensor_scalar_mul(
                                ob[:psz, j, dc * 512:(dc + 1) * 512],
                                ps2[dc][:psz, :], cw_sb[:psz, tok_i:tok_i + 1])
                    if gfull:
                        nc.sync.dma_start(
                            y_o[tok0 + g * 128:tok0 + (g + gs) * 128, :]
                            .rearrange("(s p) d -> p s d", p=128),
                            ob[:])
                    else:
                        psz = blk - g * 128
                        nc.sync.dma_start(
                            y_o[bass.ds(tok0 + g * 128, psz), :],
                            ob[:psz, 0, :])
                tok0 += blk

    nc.compile()
    return nc


# ---------------------------------------------------------------- host
def kernel(x, gate_w, w1, b1, w2, b2, gamma, beta):
    x = np.asarray(x, dtype=np.float32)
    gate_w = np.asarray(gate_w, dtype=np.float32)
    w1 = np.asarray(w1, dtype=np.float32)
    b1 = np.asarray(b1, dtype=np.float32)
    w2 = np.asarray(w2, dtype=np.float32)
    b2 = np.asarray(b2, dtype=np.float32)
    gamma = np.asarray(gamma, dtype=np.float32)
    beta = np.asarray(beta, dtype=np.float32)

    xt = np.ascontiguousarray(x.reshape(T, D))

    # ---- phase 1: LN + router ----
    affine = not (np.all(gamma == 1.0) and np.all(beta == 0.0))
    key1 = ("p1", affine)
    if key1 not in _cache:
        _cache[key1] = build_phase1(affine)
    nc1 = _cache[key1]
    in_maps1 = []
    for c in range(N_CORES):
        sl = xt[c * TC:(c + 1) * TC]
        m = {"xr": np.ascontiguousarray(sl).astype(ml_dtypes.bfloat16),
             "xT": np.ascontiguousarray(sl.T),
             "gate_w": gate_w}
        if affine:
            m["gamma_r"] = np.ascontiguousarray(gamma.reshape(KT, 128).T)
            m["beta_r"] = np.ascontiguousarray(beta.reshape(KT, 128).T)
            m["gb"] = np.ascontiguousarray(np.broadcast_to(gamma, (128, D)))
            m["bb"] = np.ascontiguousarray(np.broadcast_to(beta, (128, D)))
        in_maps1.append(m)
    res1 = run_bass_kernel_spmd(nc1, in_maps1, list(range(N_CORES)))
    LAST_RESULTS["p1"] = res1
    xn_full = np.concatenate([res1.results[c]["xn"] for c in range(N_CORES)],
                             axis=0)             # [T, D] bf16 rows
    cw_full = np.concatenate([res1.results[c]["cw"] for c in range(N_CORES)],
                             axis=0)             # [T, E] f32

    # ---- host dispatch: gather tokens by expert ----
    idxs = [np.nonzero(cw_full[:, e] != 0.0)[0] for e in range(E)]
    counts = [len(ix) for ix in idxs]
    C = max(128, ((max(counts) + 127) // 128) * 128)
    CR = C // 128

    if ("p2", C) not in _cache:
        _cache[("p2", C)] = build_phase2(C)
    nc2 = _cache[("p2", C)]

    w1_b = w1.astype(ml_dtypes.bfloat16)
    w2_8 = np.clip(w2 * W2_SCALE, -240, 240).astype(ml_dtypes.float8_e4m3fn)
    in_maps2 = []
    for e in range(E):
        ix = idxs[e]
        xnT_e = np.zeros((D, C), dtype=ml_dtypes.bfloat16)
        xnT_e[:, :counts[e]] = xn_full[ix].T
        cw_e = np.zeros((CR * 128,), dtype=np.float32)
        cw_e[:counts[e]] = cw_full[ix, e] / W2_SCALE
        in_maps2.append({
            "xnT": xnT_e,
            "w1": np.ascontiguousarray(w1_b[e]),
            "w2": np.ascontiguousarray(w2_8[e]),
            "b1r": np.ascontiguousarray(b1[e].reshape(HT, 128).T),
            "cwr": np.ascontiguousarray(cw_e.reshape(CR, 128).T),
        })
    res2 = run_bass_kernel_spmd(nc2, in_maps2, list(range(N_CORES)))
    LAST_RESULTS["p2"] = res2

    # ---- host combine: scatter-add + residual (+ per-expert b2, zero here)
    out = xt.copy()
    b2_any = bool(np.any(b2))
    for e in range(E):
        contrib = res2.results[e]["y"][:counts[e]].astype(np.float32)
        if b2_any:
            contrib = contrib + cw_full[idxs[e], e][:, None] * b2[e][None, :]
        out[idxs[e]] += contrib
    return out.reshape(B, L, D)

